# revision 15
# baseline (speedup 1.0000x reference)
"""Trainium2 Bass kernel for the Context Encoder problem:

    ce  = c2e_weight[nodes]            # [N, 128] embedding gather
    h   = relu(ce @ w1.T + b1)         # [N, 128]
    out = relu(h @ w2.T + b2)          # [N, 128]

Strategy (8 NeuronCores, vocab-range sharding):
  200000 node ids over a 100000-row vocab saturate every vocab window,
  so transforming the table itself is less work than gathering per-node
  rows.  Core i streams its host-pre-transposed (d-major) window
  [128, 12544] and computes T2 = relu(relu(win@w1.T+b1)@w2.T+b2) for
  every window row; the host maps node positions to rows (out =
  T2[nodes]) as the unshard step.

  Perf structure (memory regime, ~360GB/s/core HBM):
  - bf16 on the wire and through the PE: halves HBM traffic, and the
    PE runs 1 cycle/row instead of fp32's 4.  PSUM stays fp32.
  - Groups of 8 blocks (1024 cols, 2 PSUM banks) flow through a
    software pipeline with mm1 issued one group ahead, so the PE never
    stalls on the relu_h -> mm2 dependency.
  - relu+bias is one fused instruction per group per layer, alternated
    ACT/DVE (Pool has no PSUM access) so both engines stay under the
    PE pace.
  - Weights + output DMAs ride the Scalar-engine HWDGE queue; the
    table input stream rides the SP queue, fully buffered in SBUF so
    the DMA engines are never descriptor-starved.  A 1-block first
    chunk gets the PE started as early as possible; a 1-block last
    chunk keeps the output tail short.
"""

import sys

for _p in ("/opt/trn_rl_repo",):
    if _p not in sys.path:
        sys.path.insert(0, _p)

import ml_dtypes
import numpy as np

import concourse.bass as bass
import concourse.mybir as mybir
from concourse import bacc
from concourse.bass_utils import run_bass_kernel_spmd
from concourse.tile import TileContext

P = 128
D = 128
N_CORES = 8
VOCAB = 100000
RANGE = VOCAB // N_CORES   # 12500 vocab rows owned per core
BLOCKS = 98                # 12544 rows processed per core (128*98)
GROUP = 8                  # blocks per relu group (1024 cols, 2 PSUM banks)
MMW = 512                  # matmul free width (1 PSUM bank)

# input DMA chunks (blocks): tiny first chunks prime the compute
# pipeline, then steady 16-block transfers (DMA triggers cost ~600ns
# of issuing-engine time each, and a group's matmul waits on its whole
# chunk — so chunks grow as the pipeline fills).
CHUNKS = [1, 4, 8, 16, 16, 16, 16, 21]
assert sum(CHUNKS) == BLOCKS

BF16 = ml_dtypes.bfloat16


def build_nc():
    f32 = mybir.dt.float32
    bf16 = mybir.dt.bfloat16
    nc = bacc.Bacc("TRN2", target_bir_lowering=False, debug=False,
                   num_devices=N_CORES)

    rows = BLOCKS * P
    tsl_t = nc.dram_tensor("tslice", [P, rows], bf16,
                           kind="ExternalInput").ap()
    wb_t = nc.dram_tensor("wb", [D, 2 * D], bf16, kind="ExternalInput").ap()
    b12_t = nc.dram_tensor("b12", [P, 2], f32, kind="ExternalInput").ap()
    out_t = nc.dram_tensor("out", [P, rows], bf16,
                           kind="ExternalOutput").ap()

    gw_max = GROUP * D  # 1024

    # groups: (chunk_idx, offset within chunk tile, width, global col off)
    groups = []
    r0 = 0
    for ci, cb in enumerate(CHUNKS):
        for g0 in range(0, cb, GROUP):
            gb = min(GROUP, cb - g0)
            groups.append((ci, g0 * D, gb * D, r0 + g0 * D))
        r0 += cb * D
    n_g = len(groups)

    # output DMA batches: consecutive groups, <= 16 blocks per batch;
    # the last two groups ship as singleton batches so the tail DMA is
    # short and starts as early as possible
    batches = []
    cur = []
    cur_w = 0
    for gi, (_, _, gw, _) in enumerate(groups):
        if cur and (cur_w + gw > 16 * D or gi >= n_g - 2):
            batches.append(cur)
            cur = []
            cur_w = 0
        cur.append(gi)
        cur_w += gw
    batches.append(cur)
    batch_of = {}          # group -> (batch_idx, last-in-batch?)
    for bi, b in enumerate(batches):
        for gi in b:
            batch_of[gi] = (bi, gi == b[-1])

    with TileContext(nc) as tc:
        with (
            tc.tile_pool(name="const", bufs=1) as cpool,
            tc.tile_pool(name="winp", bufs=1) as gpool,
            tc.tile_pool(name="hT", bufs=4) as hpool,
            tc.tile_pool(name="outs", bufs=1) as opool,
            tc.tile_pool(name="psum", bufs=4, space="PSUM") as ppool,
        ):
            # weights ride in front of the table stream on the SP queue;
            # biases take the Scalar queue in parallel
            wb_sb = cpool.tile([D, 2 * D], bf16, tag="wb")
            nc.sync.dma_start(out=wb_sb[:], in_=wb_t[:])
            b12_sb = cpool.tile([P, 2], f32, tag="b12")
            nc.scalar.dma_start(out=b12_sb[:], in_=b12_t[:])
            w1t_sb = wb_sb[:, 0:D]
            w2t_sb = wb_sb[:, D : 2 * D]
            b1_sb = b12_sb[:, 0:1]
            b2_sb = b12_sb[:, 1:2]

            # the whole input window fits in SBUF: queue every chunk's
            # DMA upfront so the engines stream back-to-back.  Each
            # ~600ns trigger serializes on its issuing engine, so the
            # early (latency-critical) chunks alternate SP / Scalar.
            win_tiles = []
            r0 = 0
            for ci, cb in enumerate(CHUNKS):
                cw = cb * D
                win = gpool.tile([P, cw], bf16, tag=f"win{ci}")
                eng = nc.scalar if ci in (2, 4, 6) else nc.sync
                eng.dma_start(out=win[:], in_=tsl_t[:, r0 : r0 + cw])
                win_tiles.append(win)
                r0 += cw

            # one SBUF tile per output batch (sum = whole window, fits)
            out_tiles = []
            for bi, b in enumerate(batches):
                bw = sum(groups[gi][2] for gi in b)
                out_tiles.append(opool.tile([P, bw], bf16,
                                            name=f"outsb{bi}",
                                            tag=f"out{bi}"))

            def relu_bias(out_ap, in_ap, bias_sb, on_act):
                if on_act:
                    nc.scalar.activation(out_ap, in_ap,
                                         mybir.ActivationFunctionType.Relu,
                                         bias=bias_sb)
                else:
                    nc.vector.tensor_scalar(
                        out=out_ap, in0=in_ap, scalar1=bias_sb,
                        scalar2=0.0, op0=mybir.AluOpType.add,
                        op1=mybir.AluOpType.max)

            h_tiles = {}

            def mm1(gi):
                ci, goff, gw, _ = groups[gi]
                win = win_tiles[ci]
                h_ps = ppool.tile([P, gw_max], f32, tag="h")
                for s in range(0, gw, MMW):
                    w = min(MMW, gw - s)
                    nc.tensor.matmul(
                        out=h_ps[:, s : s + w], lhsT=w1t_sb,
                        rhs=win[:, goff + s : goff + s + w],
                        start=True, stop=True)
                h_tiles[gi] = h_ps

            def rest(gi):
                _, _, gw, r0c = groups[gi]
                h_ps = h_tiles.pop(gi)
                hT_sb = hpool.tile([P, gw_max], bf16, tag="hT")
                relu_bias(hT_sb[:, :gw], h_ps[:, :gw], b1_sb,
                          on_act=(gi % 2 == 0))
                # mm2 overwrites the group's own h PSUM tile (free once
                # relu_h has read it) — one 4-deep PSUM rotation instead
                # of two 2-deep ones, so the mm2 <- relu_o(g-2) WAR loop
                # that stalled the PE every other group disappears
                o_ps = h_ps
                for s in range(0, gw, MMW):
                    w = min(MMW, gw - s)
                    nc.tensor.matmul(
                        out=o_ps[:, s : s + w], lhsT=w2t_sb,
                        rhs=hT_sb[:, s : s + w], start=True, stop=True)
                bi, last = batch_of[gi]
                bstart = groups[batches[bi][0]][3]
                out_sb = out_tiles[bi]
                boff = r0c - bstart
                relu_bias(out_sb[:, boff : boff + gw], o_ps[:, :gw], b2_sb,
                          on_act=(gi % 2 == 1))
                if last:
                    bw = sum(groups[g][2] for g in batches[bi])
                    # early batches queue behind the input stream on SP;
                    # the final two ride the Scalar queue so the tail
                    # isn't blocked behind queued input packets
                    eng = nc.scalar if bi >= len(batches) - 2 else nc.sync
                    eng.dma_start(out=out_t[:, bstart : bstart + bw],
                                  in_=out_sb[:])

            mm1(0)
            for gi in range(n_g):
                if gi + 1 < n_g:
                    mm1(gi + 1)
                rest(gi)

    nc.compile()
    return nc


_CACHED_NC = None
LAST_RESULTS = None


def _get_nc():
    global _CACHED_NC
    if _CACHED_NC is None:
        _CACHED_NC = build_nc()
    return _CACHED_NC


def kernel(nodes, c2e_weight, w1, b1, w2, b2):
    import os

    nodes = np.asarray(nodes).astype(np.int64)
    c2e_weight = np.asarray(c2e_weight, dtype=np.float32)
    w1 = np.asarray(w1, dtype=np.float32)
    b1 = np.asarray(b1, dtype=np.float32)
    w2 = np.asarray(w2, dtype=np.float32)
    b2 = np.asarray(b2, dtype=np.float32)

    vocab = c2e_weight.shape[0]
    assert vocab == VOCAB, vocab
    rows = BLOCKS * P  # 12544

    tableT = np.ascontiguousarray(c2e_weight.T).astype(BF16)  # [128, V]

    wb = np.ascontiguousarray(
        np.concatenate([w1.T, w2.T], axis=1)).astype(BF16)    # [128, 256]
    b12 = np.ascontiguousarray(
        np.stack([b1, b2], axis=1).astype(np.float32))        # [128, 2]

    starts = []
    in_maps = []
    for i in range(N_CORES):
        start = min(i * RANGE, vocab - rows)
        starts.append(start)
        in_maps.append({
            "tslice": np.ascontiguousarray(tableT[:, start : start + rows]),
            "wb": wb,
            "b12": b12,
        })

    nc = _get_nc()
    trace = os.environ.get("BASS_KERNEL_TRACE") == "1"
    if trace:
        try:  # tracing needs the NTFF hook; degrade silently without it
            import antenv.axon_hooks  # noqa: F401
        except ImportError:
            trace = False
    res = run_bass_kernel_spmd(nc, in_maps, core_ids=list(range(N_CORES)),
                               trace=trace)
    global LAST_RESULTS
    LAST_RESULTS = res

    # T2[v] = MLP(c2e_weight[v]) assembled from the 8 windows
    t2 = np.empty((vocab, D), dtype=np.float32)
    for i in range(N_CORES):
        dense = res.results[i]["out"]                    # [128, rows] (k, r)
        lo = i * RANGE
        hi = min((i + 1) * RANGE, vocab)
        t2[lo:hi] = dense[:, lo - starts[i] : hi - starts[i]].T

    return t2[nodes]


# revision 20
# speedup vs baseline: 1.0368x; 1.0368x over previous
"""Trainium2 Bass kernel for the Context Encoder problem:

    ce  = c2e_weight[nodes]            # [N, 128] embedding gather
    h   = relu(ce @ w1.T + b1)         # [N, 128]
    out = relu(h @ w2.T + b2)          # [N, 128]

Strategy (8 NeuronCores, vocab-range sharding):
  200000 node ids over a 100000-row vocab saturate every vocab window,
  so transforming the table itself is less work than gathering per-node
  rows.  Core i streams its host-pre-transposed (d-major) window
  [128, 12544] and computes T2 = relu(relu(win@w1.T+b1)@w2.T+b2) for
  every window row; the host maps node positions to rows (out =
  T2[nodes]) as the unshard step.

  Perf structure (memory regime, ~360GB/s/core HBM):
  - bf16 on the wire and through the PE: halves HBM traffic, and the
    PE runs 1 cycle/row instead of fp32's 4.  PSUM stays fp32.
  - Groups of 8 blocks (1024 cols, 2 PSUM banks) flow through a
    software pipeline with mm1 issued one group ahead, so the PE never
    stalls on the relu_h -> mm2 dependency.
  - relu+bias is one fused instruction per group per layer, alternated
    ACT/DVE (Pool has no PSUM access) so both engines stay under the
    PE pace.
  - Weights + output DMAs ride the Scalar-engine HWDGE queue; the
    table input stream rides the SP queue, fully buffered in SBUF so
    the DMA engines are never descriptor-starved.  A 1-block first
    chunk gets the PE started as early as possible; a 1-block last
    chunk keeps the output tail short.
"""

import sys

for _p in ("/opt/trn_rl_repo",):
    if _p not in sys.path:
        sys.path.insert(0, _p)

import ml_dtypes
import numpy as np

import concourse.bass as bass
import concourse.mybir as mybir
from concourse import bacc
from concourse.bass_utils import run_bass_kernel_spmd
from concourse.tile import TileContext

P = 128
D = 128
N_CORES = 8
VOCAB = 100000
RANGE = VOCAB // N_CORES   # 12500 vocab rows owned per core
BLOCKS = 98                # 12544 rows processed per core (128*98)
GROUP = 8                  # blocks per relu group (1024 cols, 2 PSUM banks)
MMW = 512                  # matmul free width (1 PSUM bank)

# input DMA chunks (blocks): tiny first chunks prime the compute
# pipeline, then growing transfers (DMA triggers cost ~600ns of
# issuing-engine time each, and a group's matmul waits on its whole
# chunk — so chunks grow as the pipeline fills).  Chunk 0 additionally
# carries a header: wb (256 cols) + b12 (4 cols, f32 as bf16 pairs) +
# 4 pad cols, so weights+biases+first block cost ONE DMA-completion
# latency instead of three.
CHUNKS = [1, 2, 4, 8, 16, 16, 16, 35]
assert sum(CHUNKS) == BLOCKS
HDR = 264                  # header cols prepended to chunk 0

BF16 = ml_dtypes.bfloat16


def build_nc():
    f32 = mybir.dt.float32
    bf16 = mybir.dt.bfloat16
    nc = bacc.Bacc("TRN2", target_bir_lowering=False, debug=False,
                   num_devices=N_CORES)

    rows = BLOCKS * P
    tsl_t = nc.dram_tensor("tslice", [P, HDR + rows], bf16,
                           kind="ExternalInput").ap()
    out_t = nc.dram_tensor("out", [P, rows], bf16,
                           kind="ExternalOutput").ap()

    gw_max = GROUP * D  # 1024

    # groups: (chunk_idx, offset within chunk tile, width, global col off)
    groups = []
    r0 = 0
    for ci, cb in enumerate(CHUNKS):
        for g0 in range(0, cb, GROUP):
            gb = min(GROUP, cb - g0)
            groups.append((ci, g0 * D, gb * D, r0 + g0 * D))
        r0 += cb * D
    n_g = len(groups)

    # output DMA batches: consecutive groups, <= 16 blocks per batch;
    # the last two groups ship as singleton batches so the tail DMA is
    # short and starts as early as possible
    batches = []
    cur = []
    cur_w = 0
    for gi, (_, _, gw, _) in enumerate(groups):
        if cur and (cur_w + gw > 16 * D or gi >= n_g - 2):
            batches.append(cur)
            cur = []
            cur_w = 0
        cur.append(gi)
        cur_w += gw
    batches.append(cur)
    batch_of = {}          # group -> (batch_idx, last-in-batch?)
    for bi, b in enumerate(batches):
        for gi in b:
            batch_of[gi] = (bi, gi == b[-1])

    with TileContext(nc) as tc:
        with (
            tc.tile_pool(name="const", bufs=1) as cpool,
            tc.tile_pool(name="winp", bufs=1) as gpool,
            tc.tile_pool(name="hT", bufs=4) as hpool,
            tc.tile_pool(name="outs", bufs=1) as opool,
            tc.tile_pool(name="psum", bufs=4, space="PSUM") as ppool,
        ):
            # the whole input window fits in SBUF: queue every chunk's
            # DMA upfront on SP so the engines stream back-to-back.
            # Chunk 0 carries weights+biases in its header.
            win_tiles = []
            data_off = []          # data start col within each win tile
            r0 = 0
            for ci, cb in enumerate(CHUNKS):
                cw = cb * D + (HDR if ci == 0 else 0)
                win = gpool.tile([P, cw], bf16, tag=f"win{ci}")
                nc.sync.dma_start(out=win[:], in_=tsl_t[:, r0 : r0 + cw])
                win_tiles.append(win)
                data_off.append(HDR if ci == 0 else 0)
                r0 += cw

            hdr = win_tiles[0]
            w1t_sb = hdr[:, 0:D]
            w2t_sb = hdr[:, D : 2 * D]
            b12_sb = hdr[:, 2 * D : 2 * D + 4].bitcast(f32)
            b1_sb = b12_sb[:, 0:1]
            b2_sb = b12_sb[:, 1:2]

            # one SBUF tile per output batch (sum = whole window, fits)
            out_tiles = []
            for bi, b in enumerate(batches):
                bw = sum(groups[gi][2] for gi in b)
                out_tiles.append(opool.tile([P, bw], bf16,
                                            name=f"outsb{bi}",
                                            tag=f"out{bi}"))

            def relu_bias(out_ap, in_ap, bias_sb, on_act):
                if on_act:
                    nc.scalar.activation(out_ap, in_ap,
                                         mybir.ActivationFunctionType.Relu,
                                         bias=bias_sb)
                else:
                    nc.vector.tensor_scalar(
                        out=out_ap, in0=in_ap, scalar1=bias_sb,
                        scalar2=0.0, op0=mybir.AluOpType.add,
                        op1=mybir.AluOpType.max)

            h_tiles = {}

            def mm1(gi):
                ci, goff, gw, _ = groups[gi]
                win = win_tiles[ci]
                off = goff + data_off[ci]
                h_ps = ppool.tile([P, gw_max], f32, tag="h")
                for s in range(0, gw, MMW):
                    w = min(MMW, gw - s)
                    nc.tensor.matmul(
                        out=h_ps[:, s : s + w], lhsT=w1t_sb,
                        rhs=win[:, off + s : off + s + w],
                        start=True, stop=True)
                h_tiles[gi] = h_ps

            def rest(gi):
                _, _, gw, r0c = groups[gi]
                h_ps = h_tiles.pop(gi)
                hT_sb = hpool.tile([P, gw_max], bf16, tag="hT")
                relu_bias(hT_sb[:, :gw], h_ps[:, :gw], b1_sb,
                          on_act=(gi % 2 == 0))
                # mm2 overwrites the group's own h PSUM tile (free once
                # relu_h has read it) — one 4-deep PSUM rotation instead
                # of two 2-deep ones, so the mm2 <- relu_o(g-2) WAR loop
                # that stalled the PE every other group disappears
                o_ps = h_ps
                for s in range(0, gw, MMW):
                    w = min(MMW, gw - s)
                    nc.tensor.matmul(
                        out=o_ps[:, s : s + w], lhsT=w2t_sb,
                        rhs=hT_sb[:, s : s + w], start=True, stop=True)
                bi, last = batch_of[gi]
                bstart = groups[batches[bi][0]][3]
                out_sb = out_tiles[bi]
                boff = r0c - bstart
                relu_bias(out_sb[:, boff : boff + gw], o_ps[:, :gw], b2_sb,
                          on_act=(gi % 2 == 1))
                if last:
                    bw = sum(groups[g][2] for g in batches[bi])
                    # early batches queue behind the input stream on SP;
                    # the final two ride the Scalar queue so the tail
                    # isn't blocked behind queued input packets
                    eng = nc.scalar if bi >= len(batches) - 2 else nc.sync
                    eng.dma_start(out=out_t[:, bstart : bstart + bw],
                                  in_=out_sb[:])

            mm1(0)
            for gi in range(n_g):
                if gi + 1 < n_g:
                    mm1(gi + 1)
                rest(gi)

    nc.compile()
    return nc


_CACHED_NC = None
LAST_RESULTS = None


def _get_nc():
    global _CACHED_NC
    if _CACHED_NC is None:
        _CACHED_NC = build_nc()
    return _CACHED_NC


def kernel(nodes, c2e_weight, w1, b1, w2, b2):
    import os

    nodes = np.asarray(nodes).astype(np.int64)
    c2e_weight = np.asarray(c2e_weight, dtype=np.float32)
    w1 = np.asarray(w1, dtype=np.float32)
    b1 = np.asarray(b1, dtype=np.float32)
    w2 = np.asarray(w2, dtype=np.float32)
    b2 = np.asarray(b2, dtype=np.float32)

    vocab = c2e_weight.shape[0]
    assert vocab == VOCAB, vocab
    rows = BLOCKS * P  # 12544

    tableT = np.ascontiguousarray(c2e_weight.T).astype(BF16)  # [128, V]

    wb = np.concatenate([w1.T, w2.T], axis=1).astype(BF16)    # [128, 256]
    b12 = np.stack([b1, b2], axis=1).astype(np.float32)       # [128, 2]
    b12_bf = b12.view(BF16)                                   # [128, 4] raw
    pad = np.zeros((P, HDR - 2 * D - 4), dtype=BF16)
    header = np.concatenate([wb, b12_bf, pad], axis=1)        # [128, HDR]

    starts = []
    in_maps = []
    for i in range(N_CORES):
        start = min(i * RANGE, vocab - rows)
        starts.append(start)
        in_maps.append({
            "tslice": np.ascontiguousarray(np.concatenate(
                [header, tableT[:, start : start + rows]], axis=1)),
        })

    nc = _get_nc()
    trace = os.environ.get("BASS_KERNEL_TRACE") == "1"
    if trace:
        try:  # tracing needs the NTFF hook; degrade silently without it
            import antenv.axon_hooks  # noqa: F401
        except ImportError:
            trace = False
    res = run_bass_kernel_spmd(nc, in_maps, core_ids=list(range(N_CORES)),
                               trace=trace)
    global LAST_RESULTS
    LAST_RESULTS = res

    # T2[v] = MLP(c2e_weight[v]) assembled from the 8 windows
    t2 = np.empty((vocab, D), dtype=np.float32)
    for i in range(N_CORES):
        dense = res.results[i]["out"]                    # [128, rows] (k, r)
        lo = i * RANGE
        hi = min((i + 1) * RANGE, vocab)
        t2[lo:hi] = dense[:, lo - starts[i] : hi - starts[i]].T

    return t2[nodes]


# revision 23
# speedup vs baseline: 1.0441x; 1.0070x over previous
"""Trainium2 Bass kernel for the Context Encoder problem:

    ce  = c2e_weight[nodes]            # [N, 128] embedding gather
    h   = relu(ce @ w1.T + b1)         # [N, 128]
    out = relu(h @ w2.T + b2)          # [N, 128]

Strategy (8 NeuronCores, vocab-range sharding):
  200000 node ids over a 100000-row vocab saturate every vocab window,
  so transforming the table itself is less work than gathering per-node
  rows.  Core i streams its host-pre-transposed (d-major) window
  [128, 12544] and computes T2 = relu(relu(win@w1.T+b1)@w2.T+b2) for
  every window row; the host maps node positions to rows (out =
  T2[nodes]) as the unshard step.

  Perf structure (memory regime, ~360GB/s/core HBM):
  - bf16 on the wire and through the PE: halves HBM traffic, and the
    PE runs 1 cycle/row instead of fp32's 4.  PSUM stays fp32.
  - Groups of 8 blocks (1024 cols, 2 PSUM banks) flow through a
    software pipeline with mm1 issued one group ahead, so the PE never
    stalls on the relu_h -> mm2 dependency.
  - relu+bias is one fused instruction per group per layer, alternated
    ACT/DVE (Pool has no PSUM access) so both engines stay under the
    PE pace.
  - Weights + output DMAs ride the Scalar-engine HWDGE queue; the
    table input stream rides the SP queue, fully buffered in SBUF so
    the DMA engines are never descriptor-starved.  A 1-block first
    chunk gets the PE started as early as possible; a 1-block last
    chunk keeps the output tail short.
"""

import sys

for _p in ("/opt/trn_rl_repo",):
    if _p not in sys.path:
        sys.path.insert(0, _p)

import ml_dtypes
import numpy as np

import concourse.bass as bass
import concourse.mybir as mybir
from concourse import bacc
from concourse.bass_utils import run_bass_kernel_spmd
from concourse.tile import TileContext

P = 128
D = 128
N_CORES = 8
VOCAB = 100000
RANGE = VOCAB // N_CORES   # 12500 vocab rows owned per core
BLOCKS = 98                # 12544 rows processed per core (128*98)
GROUP = 8                  # blocks per relu group (1024 cols, 2 PSUM banks)
MMW = 512                  # matmul free width (1 PSUM bank)

# input DMA chunks (blocks): tiny first chunks prime the compute
# pipeline, then growing transfers (DMA triggers cost ~600ns of
# issuing-engine time each, and a group's matmul waits on its whole
# chunk — so chunks grow as the pipeline fills).  Chunk 0 additionally
# carries a header: wb (256 cols) + b12 (4 cols, f32 as bf16 pairs) +
# 4 pad cols, so weights+biases+first block cost ONE DMA-completion
# latency instead of three.
CHUNKS = [1, 2, 4, 8, 16, 16, 16, 35]
assert sum(CHUNKS) == BLOCKS
HDR = 264                  # header cols prepended to chunk 0

BF16 = ml_dtypes.bfloat16


def build_nc():
    f32 = mybir.dt.float32
    bf16 = mybir.dt.bfloat16
    nc = bacc.Bacc("TRN2", target_bir_lowering=False, debug=False,
                   num_devices=N_CORES)

    rows = BLOCKS * P
    tsl_t = nc.dram_tensor("tslice", [P, HDR + rows], bf16,
                           kind="ExternalInput").ap()
    out_t = nc.dram_tensor("out", [P, rows], bf16,
                           kind="ExternalOutput").ap()

    gw_max = GROUP * D  # 1024

    # groups: (chunk_idx, offset within chunk tile, width, global col off)
    groups = []
    r0 = 0
    for ci, cb in enumerate(CHUNKS):
        for g0 in range(0, cb, GROUP):
            gb = min(GROUP, cb - g0)
            groups.append((ci, g0 * D, gb * D, r0 + g0 * D))
        r0 += cb * D
    n_g = len(groups)

    # output DMA batches: a DMA's descriptor generation runs ~18.5ns x
    # 128 rows ~= 2.4us serialized per queue, so mid-stream batches are
    # BIG (few descriptors); the final batch (last two groups) is small
    # and partition-split across both HWDGE queues so its desc-gen
    # latency halves — it sits on the critical tail.
    batches = []
    cur = []
    cur_w = 0
    for gi, (_, _, gw, _) in enumerate(groups):
        if cur and (cur_w + gw > 32 * D or gi >= n_g - 2):
            batches.append(cur)
            cur = []
            cur_w = 0
        cur.append(gi)
        cur_w += gw
    batches.append(cur)
    if len(batches) >= 2 and len(batches[-1]) + len(batches[-2]) <= 3:
        tail = batches.pop()
        batches[-1].extend(tail)
    batch_of = {}          # group -> (batch_idx, last-in-batch?)
    for bi, b in enumerate(batches):
        for gi in b:
            batch_of[gi] = (bi, gi == b[-1])

    with TileContext(nc) as tc:
        with (
            tc.tile_pool(name="const", bufs=1) as cpool,
            tc.tile_pool(name="winp", bufs=1) as gpool,
            tc.tile_pool(name="hT", bufs=4) as hpool,
            tc.tile_pool(name="outs", bufs=1) as opool,
            tc.tile_pool(name="psum", bufs=4, space="PSUM") as ppool,
        ):
            # pre-warm the Scalar engine's activation table (the lazy
            # ACT_TABLE_LOAD otherwise lands right on the first relu's
            # critical path, costing ~1.3us)
            warm = cpool.tile([P, 1], f32, tag="warm")
            nc.vector.memset(warm[:], 0.0)
            nc.scalar.activation(warm[:], warm[:],
                                 mybir.ActivationFunctionType.Relu, bias=0.0)

            # the whole input window fits in SBUF: queue every chunk's
            # DMA upfront so the engines stream back-to-back.  Chunk 0
            # (weights+biases header and the first block) is partition-
            # split across BOTH HWDGE queues: desc-gen runs in parallel
            # and the first matmul's data lands ~1.2us earlier.
            win_tiles = []
            data_off = []          # data start col within each win tile
            r0 = 0
            for ci, cb in enumerate(CHUNKS):
                cw = cb * D + (HDR if ci == 0 else 0)
                win = gpool.tile([P, cw], bf16, tag=f"win{ci}")
                if ci == 0:
                    nc.sync.dma_start(out=win[0:64, :],
                                      in_=tsl_t[0:64, r0 : r0 + cw])
                    nc.scalar.dma_start(out=win[64:128, :],
                                        in_=tsl_t[64:128, r0 : r0 + cw])
                else:
                    nc.sync.dma_start(out=win[:],
                                      in_=tsl_t[:, r0 : r0 + cw])
                win_tiles.append(win)
                data_off.append(HDR if ci == 0 else 0)
                r0 += cw

            hdr = win_tiles[0]
            w1t_sb = hdr[:, 0:D]
            w2t_sb = hdr[:, D : 2 * D]
            b12_sb = hdr[:, 2 * D : 2 * D + 4].bitcast(f32)
            b1_sb = b12_sb[:, 0:1]
            b2_sb = b12_sb[:, 1:2]

            # one SBUF tile per output batch (sum = whole window, fits)
            out_tiles = []
            for bi, b in enumerate(batches):
                bw = sum(groups[gi][2] for gi in b)
                out_tiles.append(opool.tile([P, bw], bf16,
                                            name=f"outsb{bi}",
                                            tag=f"out{bi}"))

            def relu_bias(out_ap, in_ap, bias_sb, on_act):
                if on_act:
                    nc.scalar.activation(out_ap, in_ap,
                                         mybir.ActivationFunctionType.Relu,
                                         bias=bias_sb)
                else:
                    nc.vector.tensor_scalar(
                        out=out_ap, in0=in_ap, scalar1=bias_sb,
                        scalar2=0.0, op0=mybir.AluOpType.add,
                        op1=mybir.AluOpType.max)

            h_tiles = {}

            def mm1(gi):
                ci, goff, gw, _ = groups[gi]
                win = win_tiles[ci]
                off = goff + data_off[ci]
                h_ps = ppool.tile([P, gw_max], f32, tag="h")
                for s in range(0, gw, MMW):
                    w = min(MMW, gw - s)
                    nc.tensor.matmul(
                        out=h_ps[:, s : s + w], lhsT=w1t_sb,
                        rhs=win[:, off + s : off + s + w],
                        start=True, stop=True)
                h_tiles[gi] = h_ps

            def rest(gi):
                _, _, gw, r0c = groups[gi]
                drain = gi >= n_g - 2   # pipeline-drain zone: halve relu
                h_ps = h_tiles.pop(gi)
                hT_sb = hpool.tile([P, gw_max], bf16, tag="hT")
                if drain and gw > MMW:
                    relu_bias(hT_sb[:, :MMW], h_ps[:, :MMW], b1_sb, True)
                    relu_bias(hT_sb[:, MMW:gw], h_ps[:, MMW:gw], b1_sb,
                              False)
                else:
                    relu_bias(hT_sb[:, :gw], h_ps[:, :gw], b1_sb,
                              on_act=(gi % 2 == 0))
                # mm2 overwrites the group's own h PSUM tile (free once
                # relu_h has read it) — one 4-deep PSUM rotation instead
                # of two 2-deep ones, so the mm2 <- relu_o(g-2) WAR loop
                # that stalled the PE every other group disappears
                o_ps = h_ps
                for s in range(0, gw, MMW):
                    w = min(MMW, gw - s)
                    nc.tensor.matmul(
                        out=o_ps[:, s : s + w], lhsT=w2t_sb,
                        rhs=hT_sb[:, s : s + w], start=True, stop=True)
                bi, last = batch_of[gi]
                bstart = groups[batches[bi][0]][3]
                out_sb = out_tiles[bi]
                boff = r0c - bstart
                if drain and gw > MMW:
                    relu_bias(out_sb[:, boff : boff + MMW], o_ps[:, :MMW],
                              b2_sb, False)
                    relu_bias(out_sb[:, boff + MMW : boff + gw],
                              o_ps[:, MMW:gw], b2_sb, True)
                else:
                    relu_bias(out_sb[:, boff : boff + gw], o_ps[:, :gw],
                              b2_sb, on_act=(gi % 2 == 1))
                if last:
                    bw = sum(groups[g][2] for g in batches[bi])
                    if bi == len(batches) - 1:
                        # final batch: partition-split across both queues
                        # so its desc-gen latency halves on the tail
                        nc.scalar.dma_start(
                            out=out_t[0:64, bstart : bstart + bw],
                            in_=out_sb[0:64, :])
                        nc.sync.dma_start(
                            out=out_t[64:128, bstart : bstart + bw],
                            in_=out_sb[64:128, :])
                    else:
                        # mid-stream batches ride SP behind the input
                        # stream — they have slack, and this keeps the
                        # Scalar engine free for relu work
                        nc.sync.dma_start(
                            out=out_t[:, bstart : bstart + bw],
                            in_=out_sb[:])

            mm1(0)
            for gi in range(n_g):
                if gi + 1 < n_g:
                    mm1(gi + 1)
                rest(gi)

    nc.compile()
    return nc


_CACHED_NC = None
LAST_RESULTS = None


def _get_nc():
    global _CACHED_NC
    if _CACHED_NC is None:
        _CACHED_NC = build_nc()
    return _CACHED_NC


def kernel(nodes, c2e_weight, w1, b1, w2, b2):
    import os

    nodes = np.asarray(nodes).astype(np.int64)
    c2e_weight = np.asarray(c2e_weight, dtype=np.float32)
    w1 = np.asarray(w1, dtype=np.float32)
    b1 = np.asarray(b1, dtype=np.float32)
    w2 = np.asarray(w2, dtype=np.float32)
    b2 = np.asarray(b2, dtype=np.float32)

    vocab = c2e_weight.shape[0]
    assert vocab == VOCAB, vocab
    rows = BLOCKS * P  # 12544

    tableT = np.ascontiguousarray(c2e_weight.T).astype(BF16)  # [128, V]

    wb = np.concatenate([w1.T, w2.T], axis=1).astype(BF16)    # [128, 256]
    b12 = np.stack([b1, b2], axis=1).astype(np.float32)       # [128, 2]
    b12_bf = b12.view(BF16)                                   # [128, 4] raw
    pad = np.zeros((P, HDR - 2 * D - 4), dtype=BF16)
    header = np.concatenate([wb, b12_bf, pad], axis=1)        # [128, HDR]

    starts = []
    in_maps = []
    for i in range(N_CORES):
        start = min(i * RANGE, vocab - rows)
        starts.append(start)
        in_maps.append({
            "tslice": np.ascontiguousarray(np.concatenate(
                [header, tableT[:, start : start + rows]], axis=1)),
        })

    nc = _get_nc()
    trace = os.environ.get("BASS_KERNEL_TRACE") == "1"
    if trace:
        try:  # tracing needs the NTFF hook; degrade silently without it
            import antenv.axon_hooks  # noqa: F401
        except ImportError:
            trace = False
    res = run_bass_kernel_spmd(nc, in_maps, core_ids=list(range(N_CORES)),
                               trace=trace)
    global LAST_RESULTS
    LAST_RESULTS = res

    # T2[v] = MLP(c2e_weight[v]) assembled from the 8 windows
    t2 = np.empty((vocab, D), dtype=np.float32)
    for i in range(N_CORES):
        dense = res.results[i]["out"]                    # [128, rows] (k, r)
        lo = i * RANGE
        hi = min((i + 1) * RANGE, vocab)
        t2[lo:hi] = dense[:, lo - starts[i] : hi - starts[i]].T

    return t2[nodes]


# revision 29
# speedup vs baseline: 1.1133x; 1.0663x over previous
"""Trainium2 Bass kernel for the Context Encoder problem:

    ce  = c2e_weight[nodes]            # [N, 128] embedding gather
    h   = relu(ce @ w1.T + b1)         # [N, 128]
    out = relu(h @ w2.T + b2)          # [N, 128]

Strategy (8 NeuronCores, vocab-range sharding):
  200000 node ids over a 100000-row vocab saturate every vocab window,
  so transforming the table itself is less work than gathering per-node
  rows.  Core i streams its host-pre-transposed (d-major) window
  [128, 12544] and computes T2 = relu(relu(win@w1.T+b1)@w2.T+b2) for
  every window row; the host maps node positions to rows (out =
  T2[nodes]) as the unshard step.

  Perf structure (memory regime, ~360GB/s/core HBM):
  - bf16 on the wire and through the PE: halves HBM traffic, and the
    PE runs 1 cycle/row instead of fp32's 4.  PSUM stays fp32.
  - Groups of 8 blocks (1024 cols, 2 PSUM banks) flow through a
    software pipeline with mm1 issued one group ahead, so the PE never
    stalls on the relu_h -> mm2 dependency.
  - relu+bias is one fused instruction per group per layer, alternated
    ACT/DVE (Pool has no PSUM access) so both engines stay under the
    PE pace.
  - Weights + output DMAs ride the Scalar-engine HWDGE queue; the
    table input stream rides the SP queue, fully buffered in SBUF so
    the DMA engines are never descriptor-starved.  A 1-block first
    chunk gets the PE started as early as possible; a 1-block last
    chunk keeps the output tail short.
"""

import sys

for _p in ("/opt/trn_rl_repo",):
    if _p not in sys.path:
        sys.path.insert(0, _p)

import ml_dtypes
import numpy as np

import concourse.bass as bass
import concourse.mybir as mybir
from concourse import bacc
from concourse.bass_utils import run_bass_kernel_spmd
from concourse.tile import TileContext

P = 128
D = 128
N_CORES = 8
VOCAB = 100000
RANGE = VOCAB // N_CORES   # 12500 vocab rows owned per core
BLOCKS = 98                # full-table fallback: 12544 rows/core
CBLOCKS = 88               # compacted (unique-rows) path: 11264 rows/core
GROUP = 8                  # blocks per relu group (1024 cols, 2 PSUM banks)
MMW = 512                  # matmul free width (1 PSUM bank)
HDR = 264                  # header cols prepended to chunk 0

BF16 = ml_dtypes.bfloat16


def _chunks(blocks):
    # input DMA chunks (blocks): tiny first chunks prime the compute
    # pipeline, then growing transfers (DMA triggers cost ~600ns of
    # issuing-engine time each, and a group's matmul waits on its whole
    # chunk — so chunks grow as the pipeline fills).  Chunk 0
    # additionally carries a header: wb (256 cols) + b12 (4 cols, f32
    # as bf16 pairs) + pad, so weights+biases+first block cost ONE
    # DMA-completion latency instead of three.
    c = [1, 2, 4, 8, 16, 16, 16, blocks - 63]
    assert sum(c) == blocks and c[-1] > 0
    return c


def build_nc(blocks):
    f32 = mybir.dt.float32
    bf16 = mybir.dt.bfloat16
    nc = bacc.Bacc("TRN2", target_bir_lowering=False, debug=False,
                   num_devices=N_CORES)

    rows = blocks * P
    CHUNKS = _chunks(blocks)
    tsl_t = nc.dram_tensor("tslice", [P, HDR + rows], bf16,
                           kind="ExternalInput").ap()
    out_t = nc.dram_tensor("out", [P, rows], bf16,
                           kind="ExternalOutput").ap()

    gw_max = GROUP * D  # 1024

    # groups: (chunk_idx, offset within chunk tile, width, global col off)
    groups = []
    r0 = 0
    for ci, cb in enumerate(CHUNKS):
        for g0 in range(0, cb, GROUP):
            gb = min(GROUP, cb - g0)
            groups.append((ci, g0 * D, gb * D, r0 + g0 * D))
        r0 += cb * D
    n_g = len(groups)

    # output DMA batches: a DMA's descriptor generation runs ~18.5ns x
    # 128 rows ~= 2.4us serialized per queue, so mid-stream batches are
    # BIG (few descriptors); the final batch (last two groups) is small
    # and partition-split across both HWDGE queues so its desc-gen
    # latency halves — it sits on the critical tail.
    batches = []
    cur = []
    cur_w = 0
    for gi, (_, _, gw, _) in enumerate(groups):
        if cur and (cur_w + gw > 32 * D or gi >= n_g - 3):
            batches.append(cur)
            cur = []
            cur_w = 0
        cur.append(gi)
        cur_w += gw
    batches.append(cur)
    n_b = len(batches)
    batch_of = {}          # group -> (batch_idx, last-in-batch?)
    for bi, b in enumerate(batches):
        for gi in b:
            batch_of[gi] = (bi, gi == b[-1])

    with TileContext(nc) as tc:
        with (
            tc.tile_pool(name="const", bufs=1) as cpool,
            tc.tile_pool(name="winp", bufs=1) as gpool,
            tc.tile_pool(name="hT", bufs=4) as hpool,
            tc.tile_pool(name="outs", bufs=1) as opool,
            tc.tile_pool(name="psum", bufs=4, space="PSUM") as ppool,
        ):
            # pre-warm the Scalar engine's activation table (the lazy
            # ACT_TABLE_LOAD otherwise lands right on the first relu's
            # critical path, costing ~1.3us)
            warm = cpool.tile([P, 1], f32, tag="warm")
            nc.vector.memset(warm[:], 0.0)
            nc.scalar.activation(warm[:], warm[:],
                                 mybir.ActivationFunctionType.Relu, bias=0.0)

            # the whole input window fits in SBUF: queue every chunk's
            # DMA upfront so the engines stream back-to-back.  Chunk 0
            # (weights+biases header and the first block) is partition-
            # split across BOTH HWDGE queues: desc-gen runs in parallel
            # and the first matmul's data lands ~1.2us earlier.
            win_tiles = []
            data_off = []          # data start col within each win tile
            r0 = 0
            for ci, cb in enumerate(CHUNKS):
                cw = cb * D + (HDR if ci == 0 else 0)
                win = gpool.tile([P, cw], bf16, tag=f"win{ci}")
                if ci == 0:
                    nc.sync.dma_start(out=win[0:64, :],
                                      in_=tsl_t[0:64, r0 : r0 + cw])
                    nc.scalar.dma_start(out=win[64:128, :],
                                        in_=tsl_t[64:128, r0 : r0 + cw])
                else:
                    nc.sync.dma_start(out=win[:],
                                      in_=tsl_t[:, r0 : r0 + cw])
                win_tiles.append(win)
                data_off.append(HDR if ci == 0 else 0)
                r0 += cw

            hdr = win_tiles[0]
            w1t_sb = hdr[:, 0:D]
            w2t_sb = hdr[:, D : 2 * D]
            b12_sb = hdr[:, 2 * D : 2 * D + 4].bitcast(f32)
            b1_sb = b12_sb[:, 0:1]
            b2_sb = b12_sb[:, 1:2]

            # one SBUF tile per output batch (sum = whole window, fits)
            out_tiles = []
            for bi, b in enumerate(batches):
                bw = sum(groups[gi][2] for gi in b)
                out_tiles.append(opool.tile([P, bw], bf16,
                                            name=f"outsb{bi}",
                                            tag=f"out{bi}"))

            def relu_bias(out_ap, in_ap, bias_sb, on_act):
                if on_act:
                    nc.scalar.activation(out_ap, in_ap,
                                         mybir.ActivationFunctionType.Relu,
                                         bias=bias_sb)
                else:
                    nc.vector.tensor_scalar(
                        out=out_ap, in0=in_ap, scalar1=bias_sb,
                        scalar2=0.0, op0=mybir.AluOpType.add,
                        op1=mybir.AluOpType.max)

            h_tiles = {}

            def mm1(gi):
                ci, goff, gw, _ = groups[gi]
                win = win_tiles[ci]
                off = goff + data_off[ci]
                h_ps = ppool.tile([P, gw_max], f32, tag="h")
                for s in range(0, gw, MMW):
                    w = min(MMW, gw - s)
                    nc.tensor.matmul(
                        out=h_ps[:, s : s + w], lhsT=w1t_sb,
                        rhs=win[:, off + s : off + s + w],
                        start=True, stop=True)
                h_tiles[gi] = h_ps

            def rest(gi):
                _, _, gw, r0c = groups[gi]
                drain = gi >= n_g - 3   # pipeline-drain zone: halve relu
                h_ps = h_tiles.pop(gi)
                hT_sb = hpool.tile([P, gw_max], bf16, tag="hT")
                if drain and gw > MMW:
                    relu_bias(hT_sb[:, :MMW], h_ps[:, :MMW], b1_sb, True)
                    relu_bias(hT_sb[:, MMW:gw], h_ps[:, MMW:gw], b1_sb,
                              False)
                else:
                    relu_bias(hT_sb[:, :gw], h_ps[:, :gw], b1_sb,
                              on_act=(gi % 2 == 0))
                # mm2 overwrites the group's own h PSUM tile (free once
                # relu_h has read it) — one 4-deep PSUM rotation instead
                # of two 2-deep ones, so the mm2 <- relu_o(g-2) WAR loop
                # that stalled the PE every other group disappears
                o_ps = h_ps
                for s in range(0, gw, MMW):
                    w = min(MMW, gw - s)
                    nc.tensor.matmul(
                        out=o_ps[:, s : s + w], lhsT=w2t_sb,
                        rhs=hT_sb[:, s : s + w], start=True, stop=True)
                bi, last = batch_of[gi]
                bstart = groups[batches[bi][0]][3]
                out_sb = out_tiles[bi]
                boff = r0c - bstart
                if drain and gw > MMW:
                    relu_bias(out_sb[:, boff : boff + MMW], o_ps[:, :MMW],
                              b2_sb, False)
                    relu_bias(out_sb[:, boff + MMW : boff + gw],
                              o_ps[:, MMW:gw], b2_sb, True)
                else:
                    relu_bias(out_sb[:, boff : boff + gw], o_ps[:, :gw],
                              b2_sb, on_act=(gi % 2 == 1))
                if last:
                    bw = sum(groups[g][2] for g in batches[bi])
                    if bi >= n_b - 3:
                        # tail batches: partition-split across both
                        # queues so each one's desc-gen latency halves
                        # (the last DMA sits on the critical tail)
                        nc.scalar.dma_start(
                            out=out_t[0:64, bstart : bstart + bw],
                            in_=out_sb[0:64, :])
                        nc.sync.dma_start(
                            out=out_t[64:128, bstart : bstart + bw],
                            in_=out_sb[64:128, :])
                    else:
                        # big mid-stream batches alternate queues so
                        # neither accumulates a desc-gen backlog behind
                        # the input stream
                        eng = nc.sync if bi % 2 == 0 else nc.scalar
                        eng.dma_start(
                            out=out_t[:, bstart : bstart + bw],
                            in_=out_sb[:])

            mm1(0)
            for gi in range(n_g):
                if gi + 1 < n_g:
                    mm1(gi + 1)
                rest(gi)

    nc.compile()
    return nc


_CACHED_NC = {}
LAST_RESULTS = None


def _get_nc(blocks):
    if blocks not in _CACHED_NC:
        _CACHED_NC[blocks] = build_nc(blocks)
    return _CACHED_NC[blocks]


def _run(in_maps):
    import os

    trace = os.environ.get("BASS_KERNEL_TRACE") == "1"
    if trace:
        try:  # tracing needs the NTFF hook; degrade silently without it
            import antenv.axon_hooks  # noqa: F401
        except ImportError:
            trace = False
    blocks = (in_maps[0]["tslice"].shape[1] - HDR) // P
    res = run_bass_kernel_spmd(_get_nc(blocks), in_maps,
                               core_ids=list(range(N_CORES)), trace=trace)
    global LAST_RESULTS
    LAST_RESULTS = res
    return res


def kernel(nodes, c2e_weight, w1, b1, w2, b2):
    nodes = np.asarray(nodes).astype(np.int64)
    c2e_weight = np.asarray(c2e_weight, dtype=np.float32)
    w1 = np.asarray(w1, dtype=np.float32)
    b1 = np.asarray(b1, dtype=np.float32)
    w2 = np.asarray(w2, dtype=np.float32)
    b2 = np.asarray(b2, dtype=np.float32)

    vocab = c2e_weight.shape[0]
    assert vocab == VOCAB, vocab

    tableT = np.ascontiguousarray(c2e_weight.T).astype(BF16)  # [128, V]

    wb = np.concatenate([w1.T, w2.T], axis=1).astype(BF16)    # [128, 256]
    b12 = np.stack([b1, b2], axis=1).astype(np.float32)       # [128, 2]
    b12_bf = b12.view(BF16)                                   # [128, 4] raw
    pad = np.zeros((P, HDR - 2 * D - 4), dtype=BF16)
    header = np.concatenate([wb, b12_bf, pad], axis=1)        # [128, HDR]

    # Compacted path: only vocab rows actually referenced by `nodes`
    # are pushed through the MLP (~86.5% of the vocab for uniform ids;
    # 8x88x128 = 90112 rows of capacity).  Falls back to transforming
    # the full table when the unique count doesn't fit.
    uniq, inv = np.unique(nodes, return_inverse=True)
    u = len(uniq)
    crows = CBLOCKS * P
    if u <= N_CORES * crows:
        tc = tableT[:, uniq]                              # [128, U] gather
        if u < N_CORES * crows:
            tc = np.concatenate(
                [tc, np.zeros((P, N_CORES * crows - u), dtype=BF16)],
                axis=1)
        in_maps = [{
            "tslice": np.ascontiguousarray(np.concatenate(
                [header, tc[:, i * crows : (i + 1) * crows]], axis=1)),
        } for i in range(N_CORES)]
        res = _run(in_maps)
        t2c = np.empty((u, D), dtype=np.float32)
        for i in range(N_CORES):
            lo = i * crows
            hi = min(lo + crows, u)
            if lo >= u:
                break
            dense = res.results[i]["out"]                 # [128, crows]
            t2c[lo:hi] = dense[:, : hi - lo].T
        return t2c[inv]

    # Full-table fallback: vocab-range sharding, T2[v] for every v
    rows = BLOCKS * P
    starts = []
    in_maps = []
    for i in range(N_CORES):
        start = min(i * RANGE, vocab - rows)
        starts.append(start)
        in_maps.append({
            "tslice": np.ascontiguousarray(np.concatenate(
                [header, tableT[:, start : start + rows]], axis=1)),
        })
    res = _run(in_maps)
    t2 = np.empty((vocab, D), dtype=np.float32)
    for i in range(N_CORES):
        dense = res.results[i]["out"]                    # [128, rows] (k, r)
        lo = i * RANGE
        hi = min((i + 1) * RANGE, vocab)
        t2[lo:hi] = dense[:, lo - starts[i] : hi - starts[i]].T

    return t2[nodes]
